# revision 6
# baseline (speedup 1.0000x reference)
"""Trainium2 Bass kernel for nn_BaseEmbedder (retrieval_knn).

For each of 4096 query embeddings: find the 5 nearest of 65536 db embeddings
(Euclidean) and produce the inverse-distance-weighted sum of their auxiliary
features.  SPMD on 8 NeuronCores: queries sharded 512/core, db+aux replicated.

Per core (512 queries = 4 q-tiles of 128 partitions), two passes of 2 q-tiles:
  - Scan (bf16): negS[q,j] = q.x_j - 0.5|x_j|^2 via K=34 augmented bf16
    matmuls (rows 32/33 carry the -0.5|x|^2 bias split hi/lo).  Single PE
    row-group; db spans of 4096 cols are streamed into SBUF once per pass
    and used by both q-tiles of the pass while resident.
  - PSUM evacuation: scalar (ACT) engine copies each [128,1024] PSUM tile
    to SBUF with bf16 downcast.  All subsequent fold work is 2-byte, which
    runs the DVE tensor_tensor ops in 2x perf mode.
  - 8-way fold: per 4096-col span, three halving max-TTs fold to 512 slots;
    slot u of span j covers db rows (8j+k)*512+u, k=0..7.  zfold[128,8192]
    bf16 covers the whole db per q-tile.
  - Candidates: max8 over zfold gives top-8 folded values; max_index
    recovers their fold slots (duplicate needles match successive distinct
    slots, so bf16 ties are safe).  Each slot maps to EIGHT db rows, all
    gathered, so no disambiguation is needed.
  - Exact refinement (f32): a host-prepared table
    row[s] = [x,aux of the 8 covered rows, then 8x |x|^2] is gathered per
    winning slot via per-partition indirect DMA.  Exact distances for all
    64 candidates are recomputed on-chip; top-5 by threshold (5th-largest
    of 2*q.x - |x|^2); weights 1/(d+eps) normalized; weighted aux sum.

The bf16 scan only nominates candidates; all selection/weight math is exact
f32, so the result matches the f32 reference to ~1e-6.
"""

import numpy as np
import ml_dtypes

from concourse import bass, mybir
from concourse.tile import TileContext
from concourse.bass_utils import run_bass_kernel_spmd

F32 = mybir.dt.float32
BF16 = mybir.dt.bfloat16
U32 = mybir.dt.uint32
I32 = mybir.dt.int32

N_CORES = 8
NQ = 4096
NDB = 65536
D = 32
DAUG = 34   # 32 dims + bias row + bias-residual row (bf16 split)
K = 5
EPS = 1e-6

NQ_CORE = NQ // N_CORES          # 512
CHUNK = 512                      # db columns per matmul (one PSUM bank)
PTILE = 2048                     # PSUM tile width (4 banks)
SPAN = 4096                      # db columns per fold span
G = 8                            # fold degree: rows per fold slot
SLOTW = SPAN // G                # 512 fold slots per span
NSPAN = NDB // SPAN              # 16 spans
FOLD_W = NDB // G                # 8192 folded columns per q-tile
NCAND = 8 * G                    # 64 candidates = 8 needles x 8 rows/slot
PV = G * 2 * D + G               # 520: 8x [x(32) aux(32)] then 8x |x|^2


def build_nc(nq_core=NQ_CORE, ndb=NDB):
    n_qt = nq_core // 128

    nc = bass.Bass()
    qT = nc.declare_dram_parameter("qT_aug", [DAUG, nq_core], BF16, isOutput=False)
    qf = nc.declare_dram_parameter("qf", [nq_core, D], F32, isOutput=False)
    qsq = nc.declare_dram_parameter("qsq", [nq_core, 1], F32, isOutput=False)
    dbT = nc.declare_dram_parameter("dbT_aug", [DAUG, ndb], BF16, isOutput=False)
    pairt = nc.declare_dram_parameter("pair_table", [ndb // G, PV], F32,
                                      isOutput=False)
    out = nc.declare_dram_parameter("out", [nq_core, D], F32, isOutput=True)

    with TileContext(nc) as tc:
        with (
            tc.tile_pool(name="db", bufs=2) as dbp,
            tc.tile_pool(name="zb", bufs=3) as zbp,
            tc.tile_pool(name="t1", bufs=2) as t1p,
            tc.tile_pool(name="t2", bufs=2) as t2p,
            tc.tile_pool(name="zf", bufs=1) as zfp,
            tc.tile_pool(name="ps", bufs=2, space="PSUM") as psp,
            tc.tile_pool(name="sm", bufs=1) as sp,
            tc.tile_pool(name="g", bufs=1) as gp,
            tc.tile_pool(name="pr", bufs=1) as prp,
        ):
            for t in range(n_qt):
                tts = (t,)
                # per-pass query tile (weights + refinement inputs)
                qts, qss, qfts, zfolds = {}, {}, {}, {}
                if True:
                    qt = sp.tile([DAUG, 128], BF16, tag=f"qt{t % 2}")
                    nc.sync.dma_start(out=qt[:], in_=qT[:, t * 128:(t + 1) * 128])
                    qts[t] = qt
                    qs = sp.tile([128, 1], F32, tag=f"qs{t % 2}")
                    nc.sync.dma_start(out=qs[:], in_=qsq[t * 128:(t + 1) * 128, :])
                    qss[t] = qs
                    qft = sp.tile([128, D], F32, tag=f"qft{t % 2}")
                    nc.sync.dma_start(out=qft[:], in_=qf[t * 128:(t + 1) * 128, :])
                    qfts[t] = qft
                    zfolds[t] = zfp.tile([128, FOLD_W], F32,
                                         name=f"zfold{t % 2}", tag=f"zf{t % 2}")

                # scan: stream db spans (re-read per q-tile pass)
                for j in range(NSPAN):
                    rhs = dbp.tile([DAUG, SPAN], BF16)
                    nc.sync.dma_start(out=rhs[:],
                                      in_=dbT[:, j * SPAN:(j + 1) * SPAN])
                    if True:
                        zb = zbp.tile([128, SPAN], BF16)
                        for p in range(SPAN // PTILE):
                            ps = psp.tile([128, PTILE], F32)
                            for m in range(PTILE // CHUNK):
                                sl = slice(m * CHUNK, (m + 1) * CHUNK)
                                nc.tensor.matmul(
                                    out=ps[:, sl], lhsT=qts[t][:],
                                    rhs=rhs[:, p * PTILE + m * CHUNK:
                                            p * PTILE + (m + 1) * CHUNK],
                                    start=True, stop=True, tile_position=(0, 0))
                            nc.scalar.copy(
                                out=zb[:, p * PTILE:(p + 1) * PTILE], in_=ps[:])
                        # fold 4096 -> 512 via three halving max-TTs (bf16 2x)
                        t1 = t1p.tile([128, SPAN // 2], BF16)
                        nc.vector.tensor_tensor(
                            out=t1[:], in0=zb[:, 0:SPAN // 2],
                            in1=zb[:, SPAN // 2:SPAN], op=mybir.AluOpType.max)
                        t2 = t2p.tile([128, SPAN // 4], BF16)
                        nc.vector.tensor_tensor(
                            out=t2[:], in0=t1[:, 0:SPAN // 4],
                            in1=t1[:, SPAN // 4:SPAN // 2],
                            op=mybir.AluOpType.max)
                        nc.vector.tensor_tensor(
                            out=zfolds[t][:, j * SLOTW:(j + 1) * SLOTW],
                            in0=t2[:, 0:SLOTW], in1=t2[:, SLOTW:2 * SLOTW],
                            op=mybir.AluOpType.max)

                # top-8 folded values + fold slots; gather; exact refinement
                for t in tts:
                    qs, qft, zfold = qss[t], qfts[t], zfolds[t]
                    w8 = sp.tile([128, 8], F32, tag=f"w8{t % 2}")
                    nc.vector.max(out=w8[:], in_=zfold[:])
                    pos = sp.tile([128, 8], U32, tag=f"pos{t % 2}")
                    nc.vector.max_index(out=pos[:], in_max=w8[:],
                                        in_values=zfold[:])
                    ji = sp.tile([128, 8], I32, tag=f"ji{t % 2}")
                    nc.vector.tensor_copy(ji[:], pos[:])
                    gxa = gp.tile([128, 8, PV], F32, tag=f"gxa{t % 2}")
                    for i in range(8):
                        nc.gpsimd.indirect_dma_start(
                            out=gxa[:, i, :], out_offset=None, in_=pairt[:],
                            in_offset=bass.IndirectOffsetOnAxis(
                                ap=ji[:, i:i + 1], axis=0))

                    # ---- exact f32 refinement over the 64 candidates ----
                    base = gxa[:, :, 0:2 * G * D].rearrange(
                        "p c (h v) -> p c h v", h=G)
                    gx = base[:, :, :, 0:D]
                    ga = base[:, :, :, D:2 * D]
                    xsq = gxa[:, :, 2 * G * D:2 * G * D + G]    # [128, 8, G]
                    pr = prp.tile([128, 8, G, D], F32, tag=f"pr{t % 2}")
                    nc.vector.tensor_tensor(
                        out=pr[:], in0=gx,
                        in1=qft[:].unsqueeze(1).unsqueeze(1)
                                  .to_broadcast([128, 8, G, D]),
                        op=mybir.AluOpType.mult)
                    dots = sp.tile([128, 8, G], F32, tag=f"dots{t % 2}")
                    nc.vector.tensor_reduce(out=dots[:], in_=pr[:],
                                            axis=mybir.AxisListType.X,
                                            op=mybir.AluOpType.add)
                    # neg2 = 2*dots - xsq  (dsq = qsq - neg2)
                    neg2 = sp.tile([128, NCAND], F32, tag=f"neg2{t % 2}")
                    nc.vector.scalar_tensor_tensor(
                        out=neg2[:].rearrange("p (c h) -> p c h", h=G),
                        in0=dots[:], scalar=2.0, in1=xsq,
                        op0=mybir.AluOpType.mult,
                        op1=mybir.AluOpType.subtract)
                    t8 = sp.tile([128, 8], F32, tag=f"t8{t % 2}")
                    nc.vector.max(out=t8[:], in_=neg2[:])
                    mask = sp.tile([128, NCAND], F32, tag=f"mask{t % 2}")
                    nc.vector.tensor_scalar(mask[:], neg2[:], t8[:, 4:5], None,
                                            op0=mybir.AluOpType.is_ge)
                    dsq = sp.tile([128, NCAND], F32, tag=f"dsq{t % 2}")
                    nc.vector.tensor_scalar(dsq[:], neg2[:], -1.0, qs[:, 0:1],
                                            op0=mybir.AluOpType.mult,
                                            op1=mybir.AluOpType.add)
                    nc.vector.tensor_scalar_max(dsq[:], dsq[:], 0.0)
                    dist = sp.tile([128, NCAND], F32, tag=f"dist{t % 2}")
                    nc.scalar.sqrt(out=dist[:], in_=dsq[:])
                    nc.vector.tensor_scalar_add(dist[:], dist[:], EPS)
                    rec = sp.tile([128, NCAND], F32, tag=f"rec{t % 2}")
                    nc.vector.reciprocal(out=rec[:], in_=dist[:])
                    wgt = sp.tile([128, NCAND], F32, tag=f"wgt{t % 2}")
                    nc.vector.tensor_tensor(out=wgt[:], in0=rec[:], in1=mask[:],
                                            op=mybir.AluOpType.mult)
                    wsum = sp.tile([128, 1], F32, tag=f"wsum{t % 2}")
                    nc.vector.tensor_reduce(out=wsum[:], in_=wgt[:],
                                            axis=mybir.AxisListType.X,
                                            op=mybir.AluOpType.add)
                    winv = sp.tile([128, 1], F32, tag=f"winv{t % 2}")
                    nc.vector.reciprocal(out=winv[:], in_=wsum[:])

                    # weighted sum of gathered aux rows
                    prod = prp.tile([128, 8, G, D], F32, tag=f"prod{t % 2}")
                    nc.vector.tensor_tensor(
                        out=prod[:], in0=ga,
                        in1=wgt[:].rearrange("p (c h) -> p c h", h=G)
                                  .unsqueeze(-1).to_broadcast([128, 8, G, D]),
                        op=mybir.AluOpType.mult)
                    acc = sp.tile([128, D], F32, tag=f"accr{t % 2}")
                    nc.vector.tensor_reduce(
                        out=acc[:],
                        in_=prod[:].rearrange("p i h a -> p a (i h)"),
                        axis=mybir.AxisListType.X, op=mybir.AluOpType.add)
                    outt = sp.tile([128, D], F32, tag=f"outt{t % 2}")
                    nc.vector.tensor_scalar(outt[:], acc[:], winv[:, 0:1], None,
                                            op0=mybir.AluOpType.mult)
                    nc.sync.dma_start(out=out[t * 128:(t + 1) * 128, :],
                                      in_=outt[:])

    split_multi_waits(nc)
    return nc


def split_multi_waits(nc):
    """The walrus build in this container supports a single sync-wait per
    instruction; Tile's tail drain carries one wait per live proc.  Split
    any multi-wait instruction into single-wait NoOps ahead of it."""
    for f in nc.m.functions:
        for blk in f.blocks:
            newinsts = []
            for ins in blk.instructions:
                si = ins.sync_info
                if si is not None and si.on_wait and len(si.on_wait) > 1:
                    waits = list(si.on_wait)
                    for k, w in enumerate(waits[:-1]):
                        nop = mybir.InstNoOp(name=f"{ins.name}-ws{k}", ins=[],
                                             outs=[])
                        nop.engine = ins.engine
                        nop.sync_info = mybir.SyncInfo(on_wait=[w], on_update=[])
                        newinsts.append(nop)
                    ins.sync_info = mybir.SyncInfo(on_wait=[waits[-1]],
                                                   on_update=list(si.on_update))
                newinsts.append(ins)
            blk.instructions = newinsts


def make_in_maps(embedding_features, db_embedding, auxiliary_features):
    q = np.ascontiguousarray(np.asarray(embedding_features, dtype=np.float32))
    db = np.ascontiguousarray(np.asarray(db_embedding, dtype=np.float32))
    aux = np.ascontiguousarray(np.asarray(auxiliary_features, dtype=np.float32))
    ndb = db.shape[0]
    nq_core = q.shape[0] // N_CORES
    bf = ml_dtypes.bfloat16
    bias = -0.5 * (db * db).sum(1)                      # exact f32
    b_hi = bias.astype(bf).astype(np.float32)
    b_lo = (bias - b_hi).astype(bf)
    dbT_aug = np.ascontiguousarray(np.concatenate(
        [db.T.astype(bf), b_hi.astype(bf)[None, :], b_lo[None, :]], axis=0,
        dtype=bf))
    # pair table: fold slot s = j*SLOTW + u covers db rows (G*j+k)*SLOTW + u
    idx = np.arange(ndb // G)
    j_i = idx // SLOTW
    u_i = idx % SLOTW
    dbsq = (db * db).sum(1)
    pair_table = np.zeros((ndb // G, PV), np.float32)
    for k in range(G):
        jm = (G * j_i + k) * SLOTW + u_i
        pair_table[:, 2 * k * D:(2 * k + 1) * D] = db[jm]
        pair_table[:, (2 * k + 1) * D:(2 * k + 2) * D] = aux[jm]
        pair_table[:, 2 * G * D + k] = dbsq[jm]
    pair_table = np.ascontiguousarray(pair_table)
    in_maps = []
    for c in range(N_CORES):
        qs = q[c * nq_core:(c + 1) * nq_core]
        qT_aug = np.ascontiguousarray(np.concatenate(
            [qs.T.astype(bf), np.ones((2, nq_core), bf)], axis=0, dtype=bf))
        qsq = np.ascontiguousarray((qs * qs).sum(1).reshape(nq_core, 1)
                                   ).astype(np.float32)
        in_maps.append({"qT_aug": qT_aug, "qf": qs, "qsq": qsq,
                        "dbT_aug": dbT_aug, "pair_table": pair_table})
    return in_maps


_NC_CACHE = {}


def get_nc(nq_core=NQ_CORE, ndb=NDB):
    key = (nq_core, ndb)
    if key not in _NC_CACHE:
        _NC_CACHE[key] = build_nc(nq_core, ndb)
    return _NC_CACHE[key]


def kernel(embedding_features, db_embedding, auxiliary_features):
    in_maps = make_in_maps(embedding_features, db_embedding, auxiliary_features)
    nc = get_nc()
    res = run_bass_kernel_spmd(nc, in_maps, list(range(N_CORES)))
    return np.concatenate([res.results[c]["out"] for c in range(N_CORES)],
                          axis=0).astype(np.float32)


# revision 7
# speedup vs baseline: 1.1687x; 1.1687x over previous
"""Trainium2 Bass kernel for nn_BaseEmbedder (retrieval_knn).

For each of 4096 query embeddings: find the 5 nearest of 65536 db embeddings
(Euclidean) and produce the inverse-distance-weighted sum of their auxiliary
features.  SPMD on 8 NeuronCores: queries sharded 512/core, db+aux replicated.

Per core (512 queries = 4 q-tiles of 128 partitions), two passes of 2 q-tiles:
  - Scan (bf16): negS[q,j] = q.x_j - 0.5|x_j|^2 via K=34 augmented bf16
    matmuls (rows 32/33 carry the -0.5|x|^2 bias split hi/lo).  Single PE
    row-group; db spans of 4096 cols are streamed into SBUF once per pass
    and used by both q-tiles of the pass while resident.
  - PSUM evacuation: scalar (ACT) engine copies each [128,1024] PSUM tile
    to SBUF with bf16 downcast.  All subsequent fold work is 2-byte, which
    runs the DVE tensor_tensor ops in 2x perf mode.
  - 8-way fold: per 4096-col span, three halving max-TTs fold to 512 slots;
    slot u of span j covers db rows (8j+k)*512+u, k=0..7.  zfold[128,8192]
    bf16 covers the whole db per q-tile.
  - Candidates: max8 over zfold gives top-8 folded values; max_index
    recovers their fold slots (duplicate needles match successive distinct
    slots, so bf16 ties are safe).  Each slot maps to EIGHT db rows, all
    gathered, so no disambiguation is needed.
  - Exact refinement (f32): a host-prepared table
    row[s] = [x,aux of the 8 covered rows, then 8x |x|^2] is gathered per
    winning slot via per-partition indirect DMA.  Exact distances for all
    64 candidates are recomputed on-chip; top-5 by threshold (5th-largest
    of 2*q.x - |x|^2); weights 1/(d+eps) normalized; weighted aux sum.

The bf16 scan only nominates candidates; all selection/weight math is exact
f32, so the result matches the f32 reference to ~1e-6.
"""

import numpy as np
import ml_dtypes

from concourse import bass, mybir
from concourse.tile import TileContext
from concourse.bass_utils import run_bass_kernel_spmd

F32 = mybir.dt.float32
BF16 = mybir.dt.bfloat16
U32 = mybir.dt.uint32
I32 = mybir.dt.int32

N_CORES = 8
NQ = 4096
NDB = 65536
D = 32
DAUG = 34   # 32 dims + bias row + bias-residual row (bf16 split)
K = 5
EPS = 1e-6

NQ_CORE = NQ // N_CORES          # 512
CHUNK = 512                      # db columns per matmul (one PSUM bank)
PTILE = 2048                     # PSUM tile width (4 banks)
SPAN = 4096                      # db columns per fold span
G = 8                            # fold degree: rows per fold slot
SLOTW = SPAN // G                # 512 fold slots per span
NSPAN = NDB // SPAN              # 16 spans
FOLD_W = NDB // G                # 8192 folded columns per q-tile
NCAND = 8 * G                    # 64 candidates = 8 needles x 8 rows/slot
PV = G * 2 * D + G               # 520: 8x [x(32) aux(32)] then 8x |x|^2


def build_nc(nq_core=NQ_CORE, ndb=NDB):
    n_qt = nq_core // 128

    nc = bass.Bass()
    qT = nc.declare_dram_parameter("qT_aug", [DAUG, nq_core], BF16, isOutput=False)
    qf = nc.declare_dram_parameter("qf", [nq_core, D], F32, isOutput=False)
    qsq = nc.declare_dram_parameter("qsq", [nq_core, 1], F32, isOutput=False)
    dbT = nc.declare_dram_parameter("dbT_aug", [DAUG, ndb], BF16, isOutput=False)
    pairt = nc.declare_dram_parameter("pair_table", [ndb // G, PV], F32,
                                      isOutput=False)
    out = nc.declare_dram_parameter("out", [nq_core, D], F32, isOutput=True)

    with TileContext(nc) as tc:
        with (
            tc.tile_pool(name="db", bufs=3) as dbp,
            tc.tile_pool(name="zb", bufs=3) as zbp,
            tc.tile_pool(name="t1", bufs=2) as t1p,
            tc.tile_pool(name="t2", bufs=2) as t2p,
            tc.tile_pool(name="zf", bufs=1) as zfp,
            tc.tile_pool(name="ps", bufs=2, space="PSUM") as psp,
            tc.tile_pool(name="sm", bufs=1) as sp,
            tc.tile_pool(name="g", bufs=1) as gp,
            tc.tile_pool(name="pr", bufs=1) as prp,
        ):
            for pas in range(n_qt // 2):
                tts = (2 * pas, 2 * pas + 1)
                # per-pass query tiles (weights + refinement inputs)
                qts, qss, qfts, zfolds = {}, {}, {}, {}
                for t in tts:
                    qt = sp.tile([DAUG, 128], BF16, tag=f"qt{t % 2}")
                    nc.sync.dma_start(out=qt[:], in_=qT[:, t * 128:(t + 1) * 128])
                    qts[t] = qt
                    qs = sp.tile([128, 1], F32, tag=f"qs{t % 2}")
                    nc.sync.dma_start(out=qs[:], in_=qsq[t * 128:(t + 1) * 128, :])
                    qss[t] = qs
                    qft = sp.tile([128, D], F32, tag=f"qft{t % 2}")
                    nc.sync.dma_start(out=qft[:], in_=qf[t * 128:(t + 1) * 128, :])
                    qfts[t] = qft
                    zfolds[t] = zfp.tile([128, FOLD_W], F32,
                                         name=f"zfold{t % 2}", tag=f"zf{t % 2}")

                # scan: stream db spans once, use for both q-tiles of the pass
                for j in range(NSPAN):
                    rhs = dbp.tile([DAUG, SPAN], BF16)
                    nc.sync.dma_start(out=rhs[:],
                                      in_=dbT[:, j * SPAN:(j + 1) * SPAN])
                    for t in tts:
                        zb = zbp.tile([128, SPAN], BF16)
                        for p in range(SPAN // PTILE):
                            ps = psp.tile([128, PTILE], F32)
                            for m in range(PTILE // CHUNK):
                                sl = slice(m * CHUNK, (m + 1) * CHUNK)
                                nc.tensor.matmul(
                                    out=ps[:, sl], lhsT=qts[t][:],
                                    rhs=rhs[:, p * PTILE + m * CHUNK:
                                            p * PTILE + (m + 1) * CHUNK],
                                    start=True, stop=True, tile_position=(0, 0))
                            nc.scalar.copy(
                                out=zb[:, p * PTILE:(p + 1) * PTILE], in_=ps[:])
                        # fold 4096 -> 512 via three halving max-TTs (bf16 2x)
                        t1 = t1p.tile([128, SPAN // 2], BF16)
                        nc.vector.tensor_tensor(
                            out=t1[:], in0=zb[:, 0:SPAN // 2],
                            in1=zb[:, SPAN // 2:SPAN], op=mybir.AluOpType.max)
                        t2 = t2p.tile([128, SPAN // 4], BF16)
                        nc.vector.tensor_tensor(
                            out=t2[:], in0=t1[:, 0:SPAN // 4],
                            in1=t1[:, SPAN // 4:SPAN // 2],
                            op=mybir.AluOpType.max)
                        nc.vector.tensor_tensor(
                            out=zfolds[t][:, j * SLOTW:(j + 1) * SLOTW],
                            in0=t2[:, 0:SLOTW], in1=t2[:, SLOTW:2 * SLOTW],
                            op=mybir.AluOpType.max)

                # top-8 folded values + fold slots; gather; exact refinement
                for t in tts:
                    qs, qft, zfold = qss[t], qfts[t], zfolds[t]
                    w8 = sp.tile([128, 8], F32, tag=f"w8{t % 2}")
                    nc.vector.max(out=w8[:], in_=zfold[:])
                    pos = sp.tile([128, 8], U32, tag=f"pos{t % 2}")
                    nc.vector.max_index(out=pos[:], in_max=w8[:],
                                        in_values=zfold[:])
                    ji = sp.tile([128, 8], I32, tag=f"ji{t % 2}")
                    nc.vector.tensor_copy(ji[:], pos[:])
                    gxa = gp.tile([128, 8, PV], F32, tag=f"gxa{t % 2}")
                    for i in range(8):
                        nc.gpsimd.indirect_dma_start(
                            out=gxa[:, i, :], out_offset=None, in_=pairt[:],
                            in_offset=bass.IndirectOffsetOnAxis(
                                ap=ji[:, i:i + 1], axis=0))

                    # ---- exact f32 refinement over the 64 candidates ----
                    base = gxa[:, :, 0:2 * G * D].rearrange(
                        "p c (h v) -> p c h v", h=G)
                    gx = base[:, :, :, 0:D]
                    ga = base[:, :, :, D:2 * D]
                    xsq = gxa[:, :, 2 * G * D:2 * G * D + G]    # [128, 8, G]
                    pr = prp.tile([128, 8, G, D], F32, tag=f"pr{t % 2}")
                    nc.vector.tensor_tensor(
                        out=pr[:], in0=gx,
                        in1=qft[:].unsqueeze(1).unsqueeze(1)
                                  .to_broadcast([128, 8, G, D]),
                        op=mybir.AluOpType.mult)
                    dots = sp.tile([128, 8, G], F32, tag=f"dots{t % 2}")
                    nc.vector.tensor_reduce(out=dots[:], in_=pr[:],
                                            axis=mybir.AxisListType.X,
                                            op=mybir.AluOpType.add)
                    # neg2 = 2*dots - xsq  (dsq = qsq - neg2)
                    neg2 = sp.tile([128, NCAND], F32, tag=f"neg2{t % 2}")
                    nc.vector.scalar_tensor_tensor(
                        out=neg2[:].rearrange("p (c h) -> p c h", h=G),
                        in0=dots[:], scalar=2.0, in1=xsq,
                        op0=mybir.AluOpType.mult,
                        op1=mybir.AluOpType.subtract)
                    t8 = sp.tile([128, 8], F32, tag=f"t8{t % 2}")
                    nc.vector.max(out=t8[:], in_=neg2[:])
                    mask = sp.tile([128, NCAND], F32, tag=f"mask{t % 2}")
                    nc.vector.tensor_scalar(mask[:], neg2[:], t8[:, 4:5], None,
                                            op0=mybir.AluOpType.is_ge)
                    dsq = sp.tile([128, NCAND], F32, tag=f"dsq{t % 2}")
                    nc.vector.tensor_scalar(dsq[:], neg2[:], -1.0, qs[:, 0:1],
                                            op0=mybir.AluOpType.mult,
                                            op1=mybir.AluOpType.add)
                    nc.vector.tensor_scalar_max(dsq[:], dsq[:], 0.0)
                    dist = sp.tile([128, NCAND], F32, tag=f"dist{t % 2}")
                    nc.scalar.sqrt(out=dist[:], in_=dsq[:])
                    nc.vector.tensor_scalar_add(dist[:], dist[:], EPS)
                    rec = sp.tile([128, NCAND], F32, tag=f"rec{t % 2}")
                    nc.vector.reciprocal(out=rec[:], in_=dist[:])
                    wgt = sp.tile([128, NCAND], F32, tag=f"wgt{t % 2}")
                    nc.vector.tensor_tensor(out=wgt[:], in0=rec[:], in1=mask[:],
                                            op=mybir.AluOpType.mult)
                    wsum = sp.tile([128, 1], F32, tag=f"wsum{t % 2}")
                    nc.vector.tensor_reduce(out=wsum[:], in_=wgt[:],
                                            axis=mybir.AxisListType.X,
                                            op=mybir.AluOpType.add)
                    winv = sp.tile([128, 1], F32, tag=f"winv{t % 2}")
                    nc.vector.reciprocal(out=winv[:], in_=wsum[:])

                    # weighted sum of gathered aux rows
                    prod = prp.tile([128, 8, G, D], F32, tag=f"prod{t % 2}")
                    nc.vector.tensor_tensor(
                        out=prod[:], in0=ga,
                        in1=wgt[:].rearrange("p (c h) -> p c h", h=G)
                                  .unsqueeze(-1).to_broadcast([128, 8, G, D]),
                        op=mybir.AluOpType.mult)
                    acc = sp.tile([128, D], F32, tag=f"accr{t % 2}")
                    nc.vector.tensor_reduce(
                        out=acc[:],
                        in_=prod[:].rearrange("p i h a -> p a (i h)"),
                        axis=mybir.AxisListType.X, op=mybir.AluOpType.add)
                    outt = sp.tile([128, D], F32, tag=f"outt{t % 2}")
                    nc.vector.tensor_scalar(outt[:], acc[:], winv[:, 0:1], None,
                                            op0=mybir.AluOpType.mult)
                    nc.sync.dma_start(out=out[t * 128:(t + 1) * 128, :],
                                      in_=outt[:])

    split_multi_waits(nc)
    return nc


def split_multi_waits(nc):
    """The walrus build in this container supports a single sync-wait per
    instruction; Tile's tail drain carries one wait per live proc.  Split
    any multi-wait instruction into single-wait NoOps ahead of it."""
    for f in nc.m.functions:
        for blk in f.blocks:
            newinsts = []
            for ins in blk.instructions:
                si = ins.sync_info
                if si is not None and si.on_wait and len(si.on_wait) > 1:
                    waits = list(si.on_wait)
                    for k, w in enumerate(waits[:-1]):
                        nop = mybir.InstNoOp(name=f"{ins.name}-ws{k}", ins=[],
                                             outs=[])
                        nop.engine = ins.engine
                        nop.sync_info = mybir.SyncInfo(on_wait=[w], on_update=[])
                        newinsts.append(nop)
                    ins.sync_info = mybir.SyncInfo(on_wait=[waits[-1]],
                                                   on_update=list(si.on_update))
                newinsts.append(ins)
            blk.instructions = newinsts


def make_in_maps(embedding_features, db_embedding, auxiliary_features):
    q = np.ascontiguousarray(np.asarray(embedding_features, dtype=np.float32))
    db = np.ascontiguousarray(np.asarray(db_embedding, dtype=np.float32))
    aux = np.ascontiguousarray(np.asarray(auxiliary_features, dtype=np.float32))
    ndb = db.shape[0]
    nq_core = q.shape[0] // N_CORES
    bf = ml_dtypes.bfloat16
    bias = -0.5 * (db * db).sum(1)                      # exact f32
    b_hi = bias.astype(bf).astype(np.float32)
    b_lo = (bias - b_hi).astype(bf)
    dbT_aug = np.ascontiguousarray(np.concatenate(
        [db.T.astype(bf), b_hi.astype(bf)[None, :], b_lo[None, :]], axis=0,
        dtype=bf))
    # pair table: fold slot s = j*SLOTW + u covers db rows (G*j+k)*SLOTW + u
    idx = np.arange(ndb // G)
    j_i = idx // SLOTW
    u_i = idx % SLOTW
    dbsq = (db * db).sum(1)
    pair_table = np.zeros((ndb // G, PV), np.float32)
    for k in range(G):
        jm = (G * j_i + k) * SLOTW + u_i
        pair_table[:, 2 * k * D:(2 * k + 1) * D] = db[jm]
        pair_table[:, (2 * k + 1) * D:(2 * k + 2) * D] = aux[jm]
        pair_table[:, 2 * G * D + k] = dbsq[jm]
    pair_table = np.ascontiguousarray(pair_table)
    in_maps = []
    for c in range(N_CORES):
        qs = q[c * nq_core:(c + 1) * nq_core]
        qT_aug = np.ascontiguousarray(np.concatenate(
            [qs.T.astype(bf), np.ones((2, nq_core), bf)], axis=0, dtype=bf))
        qsq = np.ascontiguousarray((qs * qs).sum(1).reshape(nq_core, 1)
                                   ).astype(np.float32)
        in_maps.append({"qT_aug": qT_aug, "qf": qs, "qsq": qsq,
                        "dbT_aug": dbT_aug, "pair_table": pair_table})
    return in_maps


_NC_CACHE = {}


def get_nc(nq_core=NQ_CORE, ndb=NDB):
    key = (nq_core, ndb)
    if key not in _NC_CACHE:
        _NC_CACHE[key] = build_nc(nq_core, ndb)
    return _NC_CACHE[key]


def kernel(embedding_features, db_embedding, auxiliary_features):
    in_maps = make_in_maps(embedding_features, db_embedding, auxiliary_features)
    nc = get_nc()
    res = run_bass_kernel_spmd(nc, in_maps, list(range(N_CORES)))
    return np.concatenate([res.results[c]["out"] for c in range(N_CORES)],
                          axis=0).astype(np.float32)
